# revision 8
# baseline (speedup 1.0000x reference)
"""Trainium2 Bass kernel for nn_BinarizedRNN.

Math: the reference's output is out[t] = sign(hidden_t) @ sign(Wo).T where
hidden feeds the next step only through sign(hidden_t).  With l1,l2 > 0 the
SignSensitiveBatchNorm factor (s*l1 + (1-s)*l2)/sqrt(var+eps) is strictly
positive, so it never changes any sign; with sign(Wh) == I the recurrent
matmul is the identity.  The whole net collapses to

    q_t = (u'_t >= p_{t-1}),  p_t = q_t * (-2*g_{t+1}),   (elementwise)
    u'_t = x_t @ sign(Wi).T - g_t                         (one big matmul)
    out_t = (2*q_t - 1) @ sign(Wo).T

which maps to: one K~800-augmented matmul, a DVE tensor_tensor_scan(is_ge,
mult) along time for the sign recurrence, and a matmul for the output.
Data-parallel over B across 8 cores; no collectives needed (the
batch-variance is provably inert).

Modes:
  dr   (default): hi pass in bf16 (784 x-hi rows + g/reset aug + the 16
       residual rows that don't fit the 3x256 fp8 chunks), lo pass as 3
       fp8e4 DoubleRow matmuls (K=256 each) on 64*(x - bf16(x)) against
       (1/64)*sign(Wi) — DoubleRow runs ~2x bf16 rate.  Output matmul also
       fp8 DoubleRow (p values {0,-1,-2} are fp8-exact when gates are +-1).
  hilo (fallback): 2 full bf16 passes (hi/lo split of x), bf16 output mm.

Chain layout: rows are ordered (b, c) with c = 0 a reset column (u' = +BIG,
d1 = -g_1) so 4 independent b-chains of length 65 pack into one 260-column
r-tile and a single scan instruction handles all of them.
"""
import os
import numpy as np
import ml_dtypes

T, B, IN, H, OUT = 64, 256, 784, 2048, 256
EPS = 1e-5
NCORES = 8
BS = B // NCORES        # 32 batch rows per core
KAUG = IN + 2           # +g row, +reset row
CH = T + 1              # 65-column chains (reset + 64 steps)
NB = 4                  # b-chains per r-tile
RT = NB * CH            # 260
NRT = BS // NB          # 8 r-tiles per core
NHT = H // 128          # 16
NO = OUT // 128         # 2
RCOLS = BS * CH         # 2080 total row-columns per core
BIG = 1e9
F8 = ml_dtypes.float8_e4m3fn
LOSC = 64.0             # fp8 residuals shipped as 64*lo, weights as W/64

# dr mode: lo DR chunks cover x rows [0, 768); leftover x rows [768, 784)
# ride the hi pass as extra bf16 rows.
NLO = 3                 # DR chunks of K=256
LORES = IN - NLO * 256  # 16 leftover residual rows
KHI = KAUG + LORES      # 802 rows in the hi pass

# k-chunking of the hi contraction dim
def _chunks(k):
    out, k0 = [], 0
    while k0 < k:
        kn = min(128, k - k0)
        out.append((k0, kn))
        k0 += kn
    return out

KCHUNKS = _chunks(KAUG)      # 7 (hilo mode)
KC = len(KCHUNKS)
KCHUNKS_HI = _chunks(KHI)    # 7 (dr mode: 6x128 + 34)

_CACHE = {}


def _matmul_raw(nc, out, lhsT, rhs, start, stop):
    """InstMatmult for dtypes the bass wrapper doesn't allowlist (int16).
    Mirrors BassTensorEngine.matmul() for the plain (no perf_mode) case."""
    import concourse.mybir as mybir

    eng = nc.tensor
    ifmap_ap = eng.lower_ap(rhs.opt({0}), opt=False)
    weights_ap = eng.lower_ap(lhsT.opt({0}), opt=False, for_matmul_weights=True)
    out_ap = eng.lower_ap(out)

    def _round_up(size):
        for v in (32, 64, 128):
            if v >= size:
                return v
        raise AssertionError(size)

    tile_size = (_round_up(rhs.partition_size()), _round_up(out.partition_size()))
    return eng.add_instruction(
        mybir.InstMatmult(
            name=eng.bass.get_next_instruction_name(),
            replication_resolution=0,
            replication_shift_amnt=0,
            replication_num_rows=0,
            start_tensor_calc=start,
            stop_tensor_calc=stop,
            ins=[ifmap_ap, weights_ap],
            outs=[out_ap],
            perf_mode=None,
            is_transpose=None,
            ifmap_quant_offset=None,
            weights_quant_offset=None,
            bass_skip_group_check=False,
            tile_position=(lhsT.base_partition(), out.base_partition()),
            tile_size=tile_size,
        )
    )


QS = 4096.0             # i16 mode: x is shipped as rint(x * QS) in int16
RESET16 = 24576         # i16 reset row value & weight (product 6.0e8 >> |M|)


def _build(mode: str, iters: int = 1):
    """Build the SPMD Bacc module. mode in {"dr", "i16", "hilo"}."""
    import concourse.bacc as bacc
    import concourse.mybir as mybir
    import concourse.tile as tile

    f32 = mybir.dt.float32
    bf16 = mybir.dt.bfloat16
    f8e4 = mybir.dt.float8e4
    f8e5 = mybir.dt.float8e5
    i16 = mybir.dt.int16
    DR = mybir.MatmulPerfMode.DoubleRow

    GRP = int(os.environ.get("BASS_NN_GRP", "2"))   # r-tiles per group
    PADW = ((GRP * RT + 15) // 16) * 16             # p-pair slot stride (B)

    nc = bacc.Bacc(
        "TRN2", target_bir_lowering=False, debug=False, num_devices=NCORES
    )

    if mode == "dr":
        xhi_d = nc.dram_tensor("xhi", [KHI, RCOLS], bf16, kind="ExternalInput")
        wih_d = nc.dram_tensor("wih", [KHI, H], bf16, kind="ExternalInput")
        xlo_d = nc.dram_tensor("xlo", [NLO, 128, 2, RCOLS], f8e4,
                               kind="ExternalInput")
        wlo_d = nc.dram_tensor("wlo", [NLO, 128, 2, H], f8e4,
                               kind="ExternalInput")
        wo_d = nc.dram_tensor("wo", [H, OUT], f8e4, kind="ExternalInput")
    elif mode == "i16":
        xhi_d = nc.dram_tensor("xhi", [KAUG, RCOLS], i16, kind="ExternalInput")
        wih_d = nc.dram_tensor("wih", [KAUG, H], i16, kind="ExternalInput")
        wo_d = nc.dram_tensor("wo", [H, OUT], f8e5, kind="ExternalInput")
    else:
        xhi_d = nc.dram_tensor("xhi", [KAUG, RCOLS], bf16, kind="ExternalInput")
        xlo2_d = nc.dram_tensor("xlo2", [KAUG, RCOLS], bf16, kind="ExternalInput")
        wih_d = nc.dram_tensor("wih", [KAUG, H], bf16, kind="ExternalInput")
        wo_d = nc.dram_tensor("wo", [H, OUT], bf16, kind="ExternalInput")
    d1_d = nc.dram_tensor("d1", [128, RT], f32, kind="ExternalInput")
    outt_d = nc.dram_tensor("outt", [OUT, BS * T], f32, kind="ExternalOutput")

    with tile.TileContext(nc) as tc:
        import contextlib
        with (
            tc.tile_pool(name="xw", bufs=1) as xw,
            tc.tile_pool(name="ppool", bufs=20) as ppool,
            tc.tile_pool(name="stage", bufs=4) as stage,
            tc.tile_pool(name="ps1", bufs=6, space="PSUM") as ps1,
            tc.tile_pool(name="ps2", bufs=2, space="PSUM") as ps2,
            (tc.For_i(0, iters, 1) if iters > 1 else contextlib.nullcontext()),
        ):
            # ---- resident inputs ----
            hi_chunks = KCHUNKS_HI if mode == "dr" else KCHUNKS
            hi_dt = i16 if mode == "i16" else bf16
            w_tiles, x_tiles = [], []
            for ci, (k0, kn) in enumerate(hi_chunks):
                wt = xw.tile([kn, H], hi_dt, tag=f"w{ci}")
                nc.sync.dma_start(wt[:], wih_d[k0 : k0 + kn, :])
                w_tiles.append(wt)
                xt_ = xw.tile([kn, RCOLS], hi_dt, tag=f"xh{ci}")
                nc.sync.dma_start(xt_[:], xhi_d[k0 : k0 + kn, :])
                x_tiles.append(xt_)
            if mode == "dr":
                wlo_tiles, xlo_tiles = [], []
                for c in range(NLO):
                    wl = xw.tile([128, 2, H], f8e4, tag=f"wl{c}")
                    nc.sync.dma_start(wl[:], wlo_d[c])
                    wlo_tiles.append(wl)
                    xl = xw.tile([128, 2, RCOLS], f8e4, tag=f"xl{c}")
                    nc.sync.dma_start(xl[:], xlo_d[c])
                    xlo_tiles.append(xl)
                wo_t = xw.tile([128, NHT // 2, 2, OUT], f8e4, tag="wo")
                nc.sync.dma_start(
                    wo_t[:], wo_d.rearrange("(c i p) o -> p c i o", p=128, i=2)
                )
            elif mode == "i16":
                wo_t = xw.tile([128, NHT // 2, 2, OUT], f8e5, tag="wo")
                nc.sync.dma_start(
                    wo_t[:], wo_d.rearrange("(c i p) o -> p c i o", p=128, i=2)
                )
            else:
                xlo2_tiles = []
                for ci, (k0, kn) in enumerate(KCHUNKS):
                    xl = xw.tile([kn, RCOLS], bf16, tag=f"xl{ci}")
                    nc.sync.dma_start(xl[:], xlo2_d[k0 : k0 + kn, :])
                    xlo2_tiles.append(xl)
                wo_t = xw.tile([128, NHT, OUT], bf16, tag="wo")
                nc.sync.dma_start(
                    wo_t[:], wo_d.rearrange("(c p) o -> p c o", p=128)
                )
            d1_t = xw.tile([128, RT], f32, tag="d1")
            nc.sync.dma_start(d1_t[:], d1_d[:])

            # ---- main loop ----
            if mode in ("dr", "i16"):
                p8 = f8e4 if mode == "dr" else f8e5
                n_mm = len(hi_chunks) + (NLO if mode == "dr" else 0)
                for g in range(NRT // GRP):
                    rts = list(range(g * GRP, (g + 1) * GRP))
                    # p tiles stored as ht-pairs for the DoubleRow output mm:
                    # [128, 2, PADW] fp8, slot i = ht parity
                    p_pairs = [
                        ppool.tile([128, 2, PADW], p8, tag="p",
                                   name=f"p_{g}_{c}")
                        for c in range(NHT // 2)
                    ]
                    for ht in range(NHT):
                        pss = [
                            ps1.tile([128, RT], f32, tag="mm1",
                                     name=f"ps_{g}_{ht}_{j}")
                            for j in range(GRP)
                        ]
                        i = 0
                        for ci in range(len(hi_chunks)):
                            for j, rt in enumerate(rts):
                                mm_args = (
                                    pss[j][:],
                                    w_tiles[ci][:, ht * 128 : (ht + 1) * 128],
                                    x_tiles[ci][:, rt * RT : (rt + 1) * RT],
                                )
                                if mode == "i16":
                                    _matmul_raw(nc, *mm_args,
                                                start=(i == 0),
                                                stop=(i == n_mm - 1))
                                else:
                                    nc.tensor.matmul(
                                        *mm_args,
                                        start=(i == 0),
                                        stop=(i == n_mm - 1),
                                    )
                            i += 1
                        for c in range(NLO if mode == "dr" else 0):
                            for j, rt in enumerate(rts):
                                nc.tensor.matmul(
                                    pss[j][:],
                                    wlo_tiles[c][:, :, ht * 128 : (ht + 1) * 128],
                                    xlo_tiles[c][:, :, rt * RT : (rt + 1) * RT],
                                    start=(i == 0),
                                    stop=(i == n_mm - 1),
                                    perf_mode=DR,
                                )
                            i += 1
                        for j in range(GRP):
                            nc.vector.tensor_tensor_scan(
                                p_pairs[ht // 2][
                                    :, ht % 2, j * RT : (j + 1) * RT
                                ],
                                pss[j][:],
                                d1_t[:],
                                0.0,
                                mybir.AluOpType.is_ge,
                                mybir.AluOpType.mult,
                            )
                    # output matmuls: fp8 DoubleRow over ht-pairs, N=260
                    # (includes the NB reset columns; dropped at the stage
                    # copy)
                    for j in range(GRP):
                        for o in range(NO):
                            po = ps2.tile([128, RT], f32, tag="mm2",
                                          name=f"po_{g}_{j}_{o}")
                            for c in range(NHT // 2):
                                nc.tensor.matmul(
                                    po[:],
                                    wo_t[:, c, :, o * 128 : (o + 1) * 128],
                                    p_pairs[c][:, :, j * RT : (j + 1) * RT],
                                    start=(c == 0),
                                    stop=(c == NHT // 2 - 1),
                                    perf_mode=DR,
                                )
                            st = stage.tile([128, NB, T], f32, tag="st",
                                            name=f"st_{g}_{j}_{o}")
                            nc.vector.tensor_copy(
                                st[:],
                                po[:].rearrange("p (a b) -> p a b", a=NB)[
                                    :, :, 1:
                                ],
                            )
                            col = (g * GRP + j) * NB * T
                            nc.sync.dma_start(
                                outt_d[
                                    o * 128 : (o + 1) * 128, col : col + NB * T
                                ],
                                st[:].rearrange("p a b -> p (a b)"),
                            )
            else:
                n_mm = KC * 2
                for g in range(NRT // GRP):
                    rts = list(range(g * GRP, (g + 1) * GRP))
                    p_tiles = []              # one [128, GRP*NB, CH] tile per ht
                    for ht in range(NHT):
                        pss = [ps1.tile([128, RT], f32, tag="mm1",
                                        name=f"ps_{g}_{ht}_{j}")
                               for j in range(len(rts))]
                        for i, (ci, xp) in enumerate(
                            (ci, xp) for ci in range(KC) for xp in range(2)
                        ):
                            xsrc = x_tiles[ci] if xp == 0 else xlo2_tiles[ci]
                            for j, rt in enumerate(rts):
                                nc.tensor.matmul(
                                    pss[j][:],
                                    w_tiles[ci][:, ht * 128 : (ht + 1) * 128],
                                    xsrc[:, rt * RT : (rt + 1) * RT],
                                    start=(i == 0),
                                    stop=(i == n_mm - 1),
                                )
                        p = ppool.tile([128, GRP * NB, CH], bf16, tag="p")
                        for j in range(GRP):
                            pv = p[:, j * NB : (j + 1) * NB, :].rearrange(
                                "p a b -> p (a b)"
                            )
                            nc.vector.tensor_tensor_scan(
                                pv,
                                pss[j][:],
                                d1_t[:],
                                0.0,
                                mybir.AluOpType.is_ge,
                                mybir.AluOpType.mult,
                            )
                        p_tiles.append(p)
                    # output matmuls: rt-pairs -> N=512, skip reset columns
                    PW = 2 if GRP % 2 == 0 else 1
                    for pr in range(GRP // PW):
                        for o in range(NO):
                            po = ps2.tile([128, PW * NB * T], f32, tag="mm2")
                            for ht in range(NHT):
                                nc.tensor.matmul(
                                    po[:],
                                    wo_t[:, ht, o * 128 : (o + 1) * 128],
                                    p_tiles[ht][:, PW * NB * pr : PW * NB * (pr + 1), 1:],
                                    start=(ht == 0),
                                    stop=(ht == NHT - 1),
                                )
                            st = stage.tile([128, PW * NB * T], f32, tag="st")
                            nc.vector.tensor_copy(st[:], po[:])
                            col = (g * GRP + PW * pr) * NB * T
                            nc.sync.dma_start(
                                outt_d[o * 128 : (o + 1) * 128, col : col + PW * NB * T],
                                st[:],
                            )

    nc.compile()
    return nc


def _get_module(mode, iters=1):
    key = (mode, iters, os.environ.get("BASS_NN_GRP", "2"))
    if key not in _CACHE:
        _CACHE[key] = _build(mode, iters)
    return _CACHE[key]


def _fallback_numpy(x, Wi, Wh, Wo, gates, l1, l2):
    """Direct fp32 replication of the reference for degenerate inputs."""
    Wi_b = np.sign(Wi)
    Wh_b = np.sign(Wh)
    Wo_b = np.sign(Wo)
    Bn, Hn = x.shape[1], Wi.shape[0]
    h = np.zeros((Bn, Hn), dtype=np.float32)
    outs = []
    for t in range(x.shape[0]):
        hidden = x[t] @ Wi_b.T + gates[t] * (np.sign(h) @ Wh_b.T)
        hidden = np.clip(hidden, -1.0, 1.0)
        var = hidden.var(axis=0, ddof=1, keepdims=True)
        bottom = np.sqrt(var + EPS)
        s = 1.0 / (1.0 + np.exp(-10.0 * hidden))
        hidden = (hidden * s * l1 + hidden * (1.0 - s) * l2) / bottom
        outs.append(np.sign(hidden) @ Wo_b.T)
        h = hidden
    return np.stack(outs).astype(np.float32)


def _make_d1(gates):
    """d1 per chain column: c=0 -> -g_1 ; c=1..63 -> -2*g_{c+1} ; c=64 -> -2.
    Returns (d1 [128, RT], gamma [T])."""
    gamma = np.empty(T, dtype=np.float32)
    gamma[: T - 1] = gates[1:]
    gamma[T - 1] = 1.0
    dd = np.empty(CH, dtype=np.float32)
    dd[0] = -gates[0]
    dd[1:] = -2.0 * gamma
    d1 = np.tile(np.tile(dd, NB)[None, :], (128, 1)).astype(np.float32)
    return d1, gamma, dd


def _prep_in_maps(x, gates, Wi_b, Wo_b, d1, mode):
    """Per-core input maps.  X^T layout with augmentation rows and reset
    columns: [K, BS*CH].  Column order: (b, c) with c=0 reset, c>=1 ->
    timestep c-1."""
    bf = ml_dtypes.bfloat16
    in_maps = []
    if mode == "dr":
        wih = np.empty((KHI, H), dtype=np.float32)
        wih[:IN] = Wi_b.T
        wih[IN] = -1.0                      # g row
        wih[IN + 1] = 1.0                   # reset row
        wih[KAUG:] = Wi_b.T[NLO * 256 : IN]  # leftover residual rows
        wih = wih.astype(bf)
        wlo = (Wi_b.T[: NLO * 256] / LOSC).reshape(NLO, 2, 128, H)
        wlo = np.ascontiguousarray(wlo.transpose(0, 2, 1, 3)).astype(F8)
        wo_arr = np.ascontiguousarray(Wo_b.T).astype(F8)
    else:
        wih = np.empty((KAUG, H), dtype=np.float32)
        wih[:IN] = Wi_b.T
        wih[IN] = -1.0
        wih[IN + 1] = 1.0
        wih = wih.astype(bf)
        wo_arr = np.ascontiguousarray(Wo_b.T).astype(bf)
    for c in range(NCORES):
        xs = x[:, c * BS : (c + 1) * BS, :]             # [T, BS, IN]
        xa = np.zeros((KAUG, BS, CH), dtype=np.float32)
        xa[:IN, :, 1:] = xs.transpose(2, 1, 0)          # [IN, BS, T]
        xa[IN, :, 1:] = gates[None, :]                  # g_t row
        xa[IN + 1, :, 0] = BIG                          # reset row
        xa = xa.reshape(KAUG, RCOLS)
        m = {"d1": d1, "wo": wo_arr, "wih": wih}
        if mode == "dr":
            m["wlo"] = wlo
        xhi32 = xa.astype(bf).astype(np.float32)
        lo = xa[:IN] - xhi32[:IN]                       # residuals [IN, RCOLS]
        if mode == "dr":
            xhi = np.empty((KHI, RCOLS), dtype=np.float32)
            xhi[:KAUG] = xhi32
            xhi[KAUG:] = lo[NLO * 256 : IN]
            m["xhi"] = xhi.astype(bf)
            xlo = (lo[: NLO * 256] * LOSC).reshape(NLO, 2, 128, RCOLS)
            m["xlo"] = np.ascontiguousarray(xlo.transpose(0, 2, 1, 3)).astype(F8)
        else:
            m["xhi"] = xa.astype(bf)
            xlo2 = np.zeros((KAUG, RCOLS), dtype=np.float32)
            xlo2[:IN] = lo
            m["xlo2"] = xlo2.astype(bf)
        in_maps.append(m)
    return in_maps


LAST_RESULTS = None


def kernel(x, Wi, Wh, Wo, gates, l1, l2):
    global LAST_RESULTS
    x = np.asarray(x, dtype=np.float32)
    Wi = np.asarray(Wi, dtype=np.float32)
    Wh = np.asarray(Wh, dtype=np.float32)
    Wo = np.asarray(Wo, dtype=np.float32)
    gates = np.asarray(gates, dtype=np.float32)
    l1 = np.asarray(l1, dtype=np.float32)
    l2 = np.asarray(l2, dtype=np.float32)

    fast = (
        x.shape == (T, B, IN)
        and np.all(l1 > 0)
        and np.all(l2 > 0)
        and np.array_equal(np.sign(Wh), np.eye(H, dtype=np.float32))
        and np.all(gates[1:] != 0)
    )
    if not fast:
        return _fallback_numpy(x, Wi, Wh, Wo, gates, l1, l2)

    from concourse.bass_utils import run_bass_kernel_spmd

    d1, gamma, dd = _make_d1(gates)
    mode = os.environ.get("BASS_NN_MODE", "dr")
    if mode == "dr" and not np.all(dd.astype(F8).astype(np.float32) == dd):
        # p values q*dd must be exact in fp8 for the DoubleRow output mm
        mode = "hilo"
    nc = _get_module(mode)

    Wi_b = np.sign(Wi)                      # [H, IN]
    Wo_b = np.sign(Wo)                      # [OUT, H]
    colsum = Wo_b.sum(axis=1)               # [OUT]

    in_maps = _prep_in_maps(x, gates, Wi_b, Wo_b, d1, mode)
    res = run_bass_kernel_spmd(nc, in_maps, core_ids=list(range(NCORES)))
    LAST_RESULTS = res

    out = np.empty((T, B, OUT), dtype=np.float32)
    inv_gamma = (1.0 / gamma).astype(np.float32)        # [T]
    for c in range(NCORES):
        ot = res.results[c]["outt"].reshape(OUT, BS, T)
        # out[t, b, o] = -ot[o, b, t]/gamma[t] - colsum[o]
        out[:, c * BS : (c + 1) * BS, :] = (
            -ot.transpose(2, 1, 0) * inv_gamma[:, None, None]
            - colsum[None, None, :]
        )
    return out
